# revision 12
# baseline (speedup 1.0000x reference)
"""Trainium2 Bass kernel for nn_BPModel: LSTM encoder -> latent ODE (RK4) -> decoder.

Data-parallel over 8 NeuronCores: batch 4096 -> 512 per core, all parameters
replicated, everything on-chip in [feature, batch] layout.

v2 design notes (from perfetto trace of v1):
- v1 was latency-bound on the LSTM recurrence chain (~6.9us/timestep) with
  ~350ns LDWEIGHTS per matmul: fp32r matmuls must self-load weights, so the
  walrus LDW-dedup pass could not elide anything.
- Weights (stationary operands) are now fp16: half-size loads, and walrus
  elides the second load of back-to-back matmuls sharing a stationary
  (emission keeps same-weight pairs adjacent). Moving operands stay fp32r
  (1 col/cycle at free-size >= 256) for precision.
- LSTM gates PSUM: per stream one [128, 1024] tile (2 banks), regions
  [i|f|o|g] at 256-col offsets: sigmoid(i,f,o) is ONE contiguous 768-col
  Act instr. 2 streams x 2 buffers = all 8 banks -> xproj(t+1) never waits.
- Elementwise split: t1=sig_i*tanh_g on Pool (gpsimd), t2/c'/h on DVE,
  tanh on Act. Whh matmuls skipped at t=0 (h=0).
- ODE: pn/cn trunks merged (stage1 one matmul via stacked [pn1W|cn1W]
  stationary + partition-stacked relu bias; stage2 two K=64 matmuls into one
  psum tile), one [3,256] pn3 matmul, ONE merged Exp with per-partition
  scale/bias APs, K=3 selector matmuls for row broadcasts, true (unpadded)
  weight dims, 2 independent batch streams, relu2 on DVE/Pool.

Engine instructions carry a single HW sync-wait slot; a post-Tile pass
moves excess waits onto same-engine NoOps.
"""

import sys
import numpy as np

for _p in ("/opt/trn_rl_repo",):
    if _p not in sys.path:
        sys.path.insert(0, _p)

import concourse.bass as bass
import concourse.tile as tile
import concourse.mybir as mybir
import concourse.bass_utils as _bu
from concourse.bass_utils import run_bass_kernel_spmd


def _patched_bir_verify_and_optimise(tmpdir, inp="bir.json", outp="file.neff",
                                     arch=None, *, dve_root=None):
    """Same as bass_utils.bir_verify_and_optimise but with walrus LDW
    dedup enabled (redundant LDWEIGHTS elision for back-to-back matmuls
    sharing a stationary operand)."""
    cmd = [
        _bu.get_walrus_driver(),
        "--pass",
        ",".join(["birverifier", "runtime_memory_reservation", "lower_act",
                  "lower_dve", "lower_ap_offset", "codegen", "neff_packager"]),
        "-i", inp,
        "--neff-output-filename", outp,
        "--enable-birsim=true", "--mem-mode=physical", "--policy=0",
        "--enable-ldw-opt=false",
        "--assign-static-dmas-to-sp=false",
        "--dram-page-size=256", "--enable-neff-debug-info=true",
        "--jobs", "8",
        *_bu.get_walrus_args(
            _bu.get_bir_arch(tmpdir, inp) if arch is None else arch,
            tmpdir, dve_root=dve_root),
    ]
    result = _bu.run_command(cmd, cwd=tmpdir)
    if result is not None:
        from pathlib import Path
        (Path(tmpdir) / "log.txt").write_text(result.stdout)
    return f"{tmpdir}/{outp}"


_bu.bir_verify_and_optimise = _patched_bir_verify_and_optimise

F32 = mybir.dt.float32
F32R = mybir.dt.float32r
F16 = mybir.dt.float16
AF = mybir.ActivationFunctionType
ALU = mybir.AluOpType

NCORES = 8
B, T_FULL, D_IN, H, LAT = 4096, 256, 2, 128, 128
BP = B // NCORES          # 512 batch per core
BS = BP // 2              # 256 per stream
N_STEPS = 9
SXT = 16                  # t-slots per xt3 tile (x rows 0..31, ones at 32)

# gate order in PSUM regions: i, f, o, g  (pytorch packs i, f, g, o)
GATE_PERM = (0, 1, 3, 2)
# whh/xproj emission order: g first so tanh(g) unblocks earliest
CI_ORDER = (3, 0, 1, 2)

# fp16 weight tensors (stationary paired with fp16 moving operands)
_W_SPECS = [
    ("Wball", [128, SXT * 512]),
    ("fc2W", [128, 256]),
    ("W2stk", [128, 128]),     # rows 0:64 pn2W, 64:128 cn2W
    ("pn3W", [128, 3]),
    ("cn3W", [128, 128]),
    ("selS", [3, 128]),        # rows [1;1;0] -> bcast(rowA+rowB)
    ("selC", [3, 128]),        # rows [0;0;1] -> bcast(rowC)
    ("dec1bW3", [3, 128]),
    ("dec2W", [128, 64]),
    ("dec3W", [64, 2]),
]
# fp32r weights: stationary paired with f32r-bitcast moving (h, z) — the
# PE rejects mixed 32/16-bit matmul inputs
_WR_SPECS = [
    ("Whh", [128, 512]),
    ("fc1W", [128, 256]),
    ("W1stk", [128, 128]),     # [pn1W | cn1W]
    ("dec1aW", [128, 128]),
]
# f32 bias / scale tensors
_B_SPECS = [
    ("fc1b2", [128, 2]),
    ("fc2b", [128, 1]),
    ("b1stk", [128, 1]),       # [pn1b(64); cn1b(64)]
    ("pn2b", [128, 1]), ("cn2b", [128, 1]),
    ("cn3b", [128, 1]),
    ("sc3", [3, 1]),           # exp scales [1, -1, -1]
    ("b3", [3, 1]),            # exp biases [b0, -b1, -b2]
    ("b3p", [3, 1]),           # params-exp biases [b0, b1, b2]
    ("dec1b", [128, 1]),
    ("dec2b", [64, 1]),
    ("dec3b", [2, 1]),
]


def _f32(ap):
    return ap.bitcast(F32)


def _legalize_matmul_waits(nc):
    """Engine instructions carry a single HW sync-wait slot (walrus: 'Too
    many sync wait commands'). Move excess waits onto preceding NoOps on the
    same engine queue; engine FIFO order keeps correctness."""
    n_moved = 0
    for fn in nc.m.functions:
        for bb in fn.blocks:
            out = []
            for inst in bb.instructions:
                si = inst.sync_info
                if si is not None and si.on_wait and len(si.on_wait) > 1:
                    waits = list(si.on_wait)
                    for w in waits[:-1]:
                        nop = mybir.InstNoOp(
                            name=nc.get_next_instruction_name(),
                            engine=inst.engine,
                            ins=[], outs=[],
                            sync_info=mybir.SyncInfo(on_wait=[w], on_update=[]),
                        )
                        out.append(nop)
                    si.on_wait = waits[-1:]
                    n_moved += 1
                out.append(inst)
            bb.instructions[:] = out
    return n_moved


def build_program(T=T_FULL, n_steps=N_STEPS, debug=False, legalize=True):
    dt = 1.0 / n_steps
    nxt = (T + SXT - 1) // SXT
    nc = bass.Bass()
    ins = {}
    ins["xt3"] = nc.declare_dram_parameter("xt3", [128, nxt * BP], F16,
                                           isOutput=False)
    for name, shape in _W_SPECS:
        ins[name] = nc.declare_dram_parameter(name, shape, F16, isOutput=False)
    for name, shape in _WR_SPECS:
        ins[name] = nc.declare_dram_parameter(name, shape, F32R, isOutput=False)
    for name, shape in _B_SPECS:
        ins[name] = nc.declare_dram_parameter(name, shape, F32, isOutput=False)
    y_out = nc.declare_dram_parameter("y", [2, BP], F32, isOutput=True)
    if debug:
        dbg_h = nc.declare_dram_parameter("dbg_h", [128, BP], F32, isOutput=True)
        dbg_z0 = nc.declare_dram_parameter("dbg_z0", [128, BP], F32, isOutput=True)
        dbg_zT = nc.declare_dram_parameter("dbg_zT", [128, BP], F32, isOutput=True)
        dbg_pr = nc.declare_dram_parameter("dbg_pr", [3, BP], F32, isOutput=True)

    with tile.TileContext(nc) as tc:
        with (
            tc.tile_pool(name="const", bufs=1) as cp,
            tc.tile_pool(name="state", bufs=2) as st,
        ):
            sb = {}
            sb["xt3"] = cp.tile([128, nxt * BP], F16, tag="xt3", name="xt3")
            nc.sync.dma_start(sb["xt3"][:], ins["xt3"][:])
            for name, shape in _W_SPECS:
                sb[name] = cp.tile(shape, F16, tag=name, name=name)
                nc.sync.dma_start(sb[name][:], ins[name][:])
            for name, shape in _WR_SPECS:
                sb[name] = cp.tile(shape, F32R, tag=name, name=name)
                nc.sync.dma_start(sb[name][:], ins[name][:])
            for name, shape in _B_SPECS:
                sb[name] = cp.tile(shape, F32, tag=name, name=name)
                nc.sync.dma_start(sb[name][:], ins[name][:])
            par3 = cp.tile([3, BP], F16, tag="par3")

            c = []
            for s in range(2):
                ct = st.tile([128, BS], F32, tag=f"c{s}")
                nc.gpsimd.memset(ct[:], 0.0)
                c.append(ct)

            xt3 = sb["xt3"]
            Wball = sb["Wball"]
            Whh = sb["Whh"]

            # ------------------ LSTM ------------------
            # per-stream gates psum [128, 2048]: one bank per gate [i|f|o|g],
            # 256 cols used of each 512-col bank; 2 streams = 8 banks.
            # g-gate weights are doubled host-side: tanh(g) = 2*sigmoid(2g)-1,
            # so ONE sigmoid instruction covers all four gate regions and the
            # correction runs as cheap DVE ops:
            #   u = 2*sig2g - 1 ; t1 = u*sig_i ; c' = t1 + sig_f*c
            with (
                tc.tile_pool(name="psA", bufs=1, space="PSUM") as gp,
                tc.tile_pool(name="work", bufs=3) as wp,
            ):
                h = [None, None]
                for t in range(T):
                    til, slot = divmod(t, SXT)
                    first = (t == 0)
                    gates = {}
                    for s in range(2):
                        gates[s] = gp.tile([128, 2048], F32, tag=f"g{s}",
                                           name=f"g{s}_{t}")
                    for ci in range(4):
                        for s in range(2):
                            xsl = xt3[:, BP * til + BS * s
                                      : BP * til + BS * (s + 1)]
                            nc.tensor.matmul(
                                gates[s][:, 512 * ci : 512 * ci + BS],
                                Wball[:, 512 * slot + 128 * ci
                                      : 512 * slot + 128 * (ci + 1)],
                                xsl, start=True, stop=first)
                    if not first:
                        for s in range(2):
                            for ci in range(4):
                                nc.tensor.matmul(
                                    gates[s][:, 512 * ci : 512 * ci + BS],
                                    Whh[:, 128 * ci : 128 * (ci + 1)],
                                    h[s][:], start=False, stop=True)
                    sgm = {}
                    for s in range(2):
                        sgm[s] = wp.tile([128, 4 * BS], F32, tag=f"sg{s}",
                                         name=f"sg{s}_{t}")
                        ga = gates[s][:].rearrange("p (r q) -> p r q", r=4)
                        nc.scalar.activation(sgm[s][:], ga[:, 0:4, 0:BS],
                                             AF.Sigmoid)
                    t2 = {}
                    u = {}
                    t1 = {}
                    for s in range(2):
                        # off-path: t2 = sig_f * c on Pool
                        t2[s] = wp.tile([128, BS], F32, tag=f"t2{s}",
                                        name=f"t2{s}_{t}")
                        nc.gpsimd.tensor_tensor(
                            out=t2[s][:], in0=sgm[s][:, BS : 2 * BS],
                            in1=c[s][:], op=ALU.mult)
                    for s in range(2):
                        u[s] = wp.tile([128, BS], F32, tag=f"u{s}",
                                       name=f"u{s}_{t}")
                        nc.vector.tensor_scalar(
                            out=u[s][:], in0=sgm[s][:, 3 * BS : 4 * BS],
                            scalar1=2.0, scalar2=1.0,
                            op0=ALU.mult, op1=ALU.subtract)
                        t1[s] = wp.tile([128, BS], F32, tag=f"t1{s}",
                                        name=f"t1{s}_{t}")
                        nc.vector.tensor_tensor(
                            out=t1[s][:], in0=u[s][:], in1=sgm[s][:, 0:BS],
                            op=ALU.mult)
                    cn = {}
                    for s in range(2):
                        cn[s] = st.tile([128, BS], F32, tag=f"c{s}",
                                        name=f"c{s}_{t}")
                        nc.vector.tensor_tensor(
                            out=cn[s][:], in0=t1[s][:], in1=t2[s][:],
                            op=ALU.add)
                        c[s] = cn[s]
                    tct = {}
                    for s in range(2):
                        tct[s] = wp.tile([128, BS], F32, tag=f"tc{s}",
                                         name=f"tc{s}_{t}")
                        nc.scalar.activation(tct[s][:], cn[s][:], AF.Tanh)
                    for s in range(2):
                        hn_ = st.tile([128, BS], F32R, tag=f"h{s}",
                                      name=f"h{s}_{t}")
                        nc.vector.tensor_tensor(
                            out=hn_[:], in0=sgm[s][:, 2 * BS : 3 * BS],
                            in1=tct[s][:], op=ALU.mult)
                        h[s] = hn_

            # ------------- encoder fc + ODE + decoder -------------
            with (
                tc.tile_pool(name="psB", bufs=1, space="PSUM") as pb,
                tc.tile_pool(name="ow", bufs=2) as ow,
            ):
                if debug:
                    for s in range(2):
                        nc.sync.dma_start(
                            dbg_h[:, BS * s : BS * (s + 1)], _f32(h[s][:]))
                # fc1: relu(hN @ fc1W + b); j chunks of the 256-dim output
                r1 = ow.tile([128, 1024], F16, tag="r1")
                for j in range(2):
                    pfc = pb.tile([128, 512], F32, tag=f"pA{j}", name=f"pfc{j}")
                    for s in range(2):
                        nc.tensor.matmul(
                            pfc[:, BS * s : BS * (s + 1)],
                            sb["fc1W"][:, 128 * j : 128 * (j + 1)],
                            h[s][:], start=(s == 0), stop=(s == 1))
                    nc.scalar.activation(
                        r1[:, 512 * j : 512 * (j + 1)], pfc[:], AF.Relu,
                        bias=sb["fc1b2"][:, j : j + 1])
                # fc2 (no relu)
                pz = pb.tile([128, BP], F32, tag="pB0")
                nc.tensor.matmul(pz[:], sb["fc2W"][:, 0:128], r1[:, 0:512],
                                 start=True, stop=False)
                nc.tensor.matmul(pz[:], sb["fc2W"][:, 128:256], r1[:, 512:1024],
                                 start=False, stop=True)
                zs = []
                for s in range(2):
                    zt = ow.tile([128, BS], F32R, tag=f"z{s}")
                    nc.vector.tensor_scalar(
                        out=zt[:], in0=pz[:, BS * s : BS * (s + 1)],
                        scalar1=sb["fc2b"][:], scalar2=None, op0=ALU.add)
                    zs.append(zt)
                if debug:
                    for s in range(2):
                        nc.sync.dma_start(dbg_z0[:, BS * s : BS * (s + 1)], _f32(zs[s][:]))

                def odef(zin, s, first=False, ktag="k"):
                    """One odefunc eval for stream s: k = (comp + cn3b
                    - z*(Rp + 1/Rd)) / C, trunks merged. Each matmul output
                    gets its own psum bank (tags pA..pD cycle per stream)."""
                    sl = slice(BS * s, BS * (s + 1))
                    # stage 1: both trunks in one matmul (partition-stacked)
                    p1 = pb.tile([128, 512], F32, tag=f"pA{s}", name=f"p1{s}")
                    nc.tensor.matmul(p1[:, 0:BS], sb["W1stk"][:], zin[:],
                                     start=True, stop=True)
                    s1 = ow.tile([128, BS], F16, tag=f"s1_{s}")
                    nc.scalar.activation(s1[:], p1[:, 0:BS], AF.Relu,
                                         bias=sb["b1stk"][:])
                    # stage 2: two K=64 matmuls, separate banks
                    p2a = pb.tile([128, 512], F32, tag=f"pB{s}", name=f"p2a{s}")
                    nc.tensor.matmul(p2a[:, 0:BS], sb["W2stk"][0:64, :],
                                     s1[0:64, :], start=True, stop=True)
                    p2b = pb.tile([128, 512], F32, tag=f"pC{s}", name=f"p2b{s}")
                    nc.tensor.matmul(p2b[:, 0:BS], sb["W2stk"][64:128, :],
                                     s1[64:128, :], start=True, stop=True)
                    s2p = ow.tile([128, BS], F16, tag=f"s2p{s}")
                    nc.scalar.activation(s2p[:], p2a[:, 0:BS], AF.Relu,
                                         bias=sb["pn2b"][:])
                    s2c = ow.tile([128, BS], F16, tag=f"s2c{s}")
                    nc.scalar.activation(s2c[:], p2b[:, 0:BS], AF.Relu,
                                         bias=sb["cn2b"][:])
                    # stage 3: pn3 -> [3, BS] (bank A); cn3 -> [128, BS] (bank B)
                    p3 = pb.tile([128, 512], F32, tag=f"pA{s}", name=f"p3{s}")
                    pp3 = p3[0:3, 0:BS]
                    nc.tensor.matmul(pp3, sb["pn3W"][:], s2p[:],
                                     start=True, stop=True)
                    pcn = pb.tile([128, 512], F32, tag=f"pB{s}", name=f"pcn{s}")
                    nc.tensor.matmul(pcn[:, 0:BS], sb["cn3W"][:], s2c[:],
                                     start=True, stop=True)
                    # rows = [Rp; 1/Rd; 1/C] = exp(pp3 * [1,-1,-1] + [b0,-b1,-b2])
                    rows = ow.tile([3, BS], F16, tag=f"rw{s}")
                    nc.scalar.activation(rows[:], pp3, AF.Exp,
                                         bias=sb["b3"][:], scale=sb["sc3"][:])
                    if first:
                        nc.scalar.activation(par3[:, sl], pp3, AF.Exp,
                                             bias=sb["b3p"][:], scale=1.0)
                    # Sb = bcast(Rp + 1/Rd) (bank D); Cb = bcast(1/C) (bank C)
                    pbs = pb.tile([128, 512], F32, tag=f"pD{s}", name=f"pbs{s}")
                    nc.tensor.matmul(pbs[:, 0:BS], sb["selS"][:], rows[:],
                                     start=True, stop=True)
                    pbc = pb.tile([128, 512], F32, tag=f"pC{s}", name=f"pbc{s}")
                    nc.tensor.matmul(pbc[:, 0:BS], sb["selC"][:], rows[:],
                                     start=True, stop=True)
                    # k = (comp + cn3b - z*Sb) * Cb
                    d1 = ow.tile([128, BS], F32, tag=f"d1{s}")
                    nc.vector.tensor_tensor(out=d1[:], in0=_f32(zin[:]),
                                            in1=pbs[:, 0:BS], op=ALU.mult)
                    d2 = ow.tile([128, BS], F32, tag=f"d2{s}")
                    nc.vector.scalar_tensor_tensor(
                        out=d2[:], in0=pcn[:, 0:BS], scalar=sb["cn3b"][:],
                        in1=d1[:], op0=ALU.add, op1=ALU.subtract)
                    k = ow.tile([128, BS], F32, tag=ktag)
                    nc.vector.tensor_tensor(out=k[:], in0=d2[:],
                                            in1=pbc[:, 0:BS], op=ALU.mult)
                    return k

                def sttz(k_in0, scalar, ztile, tag):
                    # f32r out: (k * scalar) + z
                    o = ow.tile([128, BS], F32R, tag=tag)
                    nc.vector.scalar_tensor_tensor(
                        out=o[:], in0=k_in0[:], scalar=float(scalar),
                        in1=_f32(ztile[:]),
                        op0=ALU.mult, op1=ALU.add)
                    return o

                def sttk(in0, scalar, in1, tag):
                    # f32 out: (in0 * scalar) + in1
                    o = ow.tile([128, BS], F32, tag=tag)
                    nc.vector.scalar_tensor_tensor(
                        out=o[:], in0=in0[:], scalar=float(scalar), in1=in1[:],
                        op0=ALU.mult, op1=ALU.add)
                    return o

                def ttp(in0, in1, op, tag):
                    # f32 out on Pool
                    o = ow.tile([128, BS], F32, tag=tag)
                    nc.gpsimd.tensor_tensor(out=o[:], in0=in0[:], in1=in1[:],
                                            op=op)
                    return o

                for step in range(n_steps):
                    for s in range(2):
                        z = zs[s]
                        k1 = odef(z, s, first=(step == 0), ktag=f"k1{s}")
                        za = sttz(k1, dt / 3.0, z, f"za{s}")   # z + dt/3 k1
                        k2 = odef(za, s, ktag=f"k2{s}")
                        u1 = sttk(k1, -1.0 / 3.0, k2, f"u1{s}")  # k2 - k1/3
                        zb = sttz(u1, dt, z, f"za{s}")  # z + dt(k2 - k1/3)
                        k3 = odef(zb, s, ktag=f"k3{s}")
                        u2 = ttp(k1, k2, ALU.subtract, f"u1{s}")
                        u3 = ttp(u2, k3, ALU.add, f"u2{s}")
                        zc2 = sttz(u3, dt, z, f"za{s}")  # z + dt(k1 - k2 + k3)
                        k4 = odef(zc2, s, ktag=f"k4{s}")
                        v1 = ttp(k2, k3, ALU.add, f"u1{s}")
                        v2 = sttk(v1, 3.0, k1, f"u2{s}")  # k1 + 3(k2 + k3)
                        v3 = ttp(v2, k4, ALU.add, f"u1{s}")
                        zs[s] = sttz(v3, dt / 8.0, z, f"z{s}")  # z + dt/8 (..)

                for s in range(2):
                    sl = slice(BS * s, BS * (s + 1))
                    if debug:
                        nc.sync.dma_start(dbg_zT[:, sl], _f32(zs[s][:]))
                        if s == 0:
                            nc.sync.dma_start(dbg_pr[:].bitcast(F16)[:, 0:BP], par3[:])
                    # decoder: zc = [zT ; params]
                    pd1 = pb.tile([128, 512], F32, tag=f"pA{s}",
                                  name=f"pd1{s}")
                    nc.tensor.matmul(pd1[:, 0:BS], sb["dec1aW"][:], zs[s][:],
                                     start=True, stop=False)
                    nc.tensor.matmul(pd1[:, 0:BS], sb["dec1bW3"][:],
                                     par3[:, sl], start=False, stop=True)
                    sd1 = ow.tile([128, BS], F16, tag=f"sd1{s}")
                    nc.scalar.activation(sd1[:], pd1[:, 0:BS], AF.Relu,
                                         bias=sb["dec1b"][:])
                    pd2 = pb.tile([128, 512], F32, tag=f"pB{s}",
                                  name=f"pd2{s}")
                    nc.tensor.matmul(pd2[0:64, 0:BS], sb["dec2W"][:], sd1[:],
                                     start=True, stop=True)
                    sd2 = ow.tile([64, BS], F16, tag=f"sd2{s}")
                    nc.scalar.activation(sd2[:], pd2[0:64, 0:BS], AF.Relu,
                                         bias=sb["dec2b"][:])
                    pd3 = pb.tile([128, 512], F32, tag=f"pC{s}",
                                  name=f"pd3{s}")
                    nc.tensor.matmul(pd3[0:2, 0:BS], sb["dec3W"][:], sd2[:],
                                     start=True, stop=True)
                    yt = ow.tile([2, BS], F32, tag=f"y{s}")
                    nc.vector.tensor_scalar(out=yt[:], in0=pd3[0:2, 0:BS],
                                            scalar1=sb["dec3b"][:],
                                            scalar2=None, op0=ALU.add)
                    nc.sync.dma_start(y_out[:, sl], yt[:])

    if legalize:
        _legalize_matmul_waits(nc)
    return nc


def prep_inputs(inputs, T=T_FULL):
    """Host-side marshaling: shard x, build xt3/Wball layouts, repack weights."""
    nxt = (T + SXT - 1) // SXT
    f = lambda a: np.ascontiguousarray(a, dtype=np.float32)
    f16 = lambda a: np.ascontiguousarray(a, dtype=np.float16)
    x = f(inputs["x"])                      # [B, T, 2]
    Wih = f(inputs["lstm_Wih"])             # [2, 512]
    Whh = f(inputs["lstm_Whh"])             # [128, 512]
    bsum = f(inputs["lstm_bih"] + inputs["lstm_bhh"])   # [512]

    # permute gate chunks (i, f, g, o) -> (i, f, o, g)
    def permc(w):
        chunks = [w[..., 128 * cc : 128 * (cc + 1)] for cc in GATE_PERM]
        return np.concatenate(chunks, axis=-1)

    Wih_p, Whh_p, bsum_p = permc(Wih), permc(Whh), permc(bsum)
    # g-gate doubled: tanh(g) computed as 2*sigmoid(2g)-1 on-chip
    Wih_p[:, 384:512] *= 2.0
    Whh_p[:, 384:512] *= 2.0
    bsum_p[384:512] *= 2.0

    # Wball: [128, SXT*512]; slot s: rows 2s,2s+1 = Wih rows, row 32 = bias
    Wball = np.zeros((128, SXT * 512), dtype=np.float32)
    for s in range(SXT):
        Wball[2 * s, 512 * s : 512 * (s + 1)] = Wih_p[0]
        Wball[2 * s + 1, 512 * s : 512 * (s + 1)] = Wih_p[1]
        Wball[32, 512 * s : 512 * (s + 1)] = bsum_p

    # xt3 per core: [128, nxt*BP]; tile t//SXT, x rows 2(t%SXT), ones row 32
    xt3_all = np.zeros((NCORES, 128, nxt * BP), dtype=np.float16)
    xs = x.reshape(NCORES, BP, T, 2)
    for core in range(NCORES):
        xc = xs[core]                       # [BP, T, 2]
        for t in range(T):
            til, slot = divmod(t, SXT)
            col0 = BP * til
            xt3_all[core, 2 * slot, col0 : col0 + BP] = xc[:, t, 0]
            xt3_all[core, 2 * slot + 1, col0 : col0 + BP] = xc[:, t, 1]
        xt3_all[core, 32, :] = 1.0

    fc1_b = f(inputs["fc1_b"])
    fc2_W = f(inputs["fc2_W"])
    pn3_b = f(inputs["pn3_b"])
    dec1_W = f(inputs["dec1_W"])            # [131, 128]

    selS = np.zeros((3, 128), dtype=np.float32)
    selS[0, :] = 1.0
    selS[1, :] = 1.0
    selC = np.zeros((3, 128), dtype=np.float32)
    selC[2, :] = 1.0

    common = {
        "Wball": f16(Wball),
        "Whh": f(Whh_p),
        "fc1W": f(inputs["fc1_W"]),
        "fc1b2": f(fc1_b.reshape(2, 128).T),
        "fc2W": f16(np.concatenate([fc2_W[0:128], fc2_W[128:256]], axis=1)),
        "fc2b": f(inputs["fc2_b"][:, None]),
        "W1stk": f(np.concatenate(
            [inputs["pn1_W"], inputs["cn1_W"]], axis=1)),   # [128, 64+64]
        "b1stk": f(np.concatenate(
            [inputs["pn1_b"], inputs["cn1_b"]])[:, None]),
        "W2stk": f16(np.concatenate(
            [inputs["pn2_W"], inputs["cn2_W"]], axis=0)),  # [128, 128]
        "pn2b": f(inputs["pn2_b"][:, None]),
        "cn2b": f(inputs["cn2_b"][:, None]),
        "pn3W": f16(inputs["pn3_W"]),        # [128, 3]
        "cn3W": f16(inputs["cn3_W"]),        # [128, 128]
        "cn3b": f(inputs["cn3_b"][:, None]),
        "sc3": np.array([[1.0], [-1.0], [-1.0]], dtype=np.float32),
        "b3": np.array([[pn3_b[0]], [-pn3_b[1]], [-pn3_b[2]]],
                       dtype=np.float32),
        "b3p": f(pn3_b[:, None]),
        "selS": f16(selS),
        "selC": f16(selC),
        "dec1aW": f(dec1_W[0:128]),
        "dec1bW3": f16(dec1_W[128:131]),
        "dec1b": f(inputs["dec1_b"][:, None]),
        "dec2W": f16(inputs["dec2_W"]),      # [128, 64]
        "dec2b": f(inputs["dec2_b"][:, None]),
        "dec3W": f16(inputs["dec3_W"]),      # [64, 2]
        "dec3b": f(inputs["dec3_b"][:, None]),
    }

    in_maps = []
    for core in range(NCORES):
        m = dict(common)
        m["xt3"] = xt3_all[core]
        in_maps.append(m)
    return in_maps


_PROGRAM = None


def get_program():
    global _PROGRAM
    if _PROGRAM is None:
        _PROGRAM = build_program()
    return _PROGRAM


def run(inputs, **kwargs):
    nc = get_program()
    in_maps = prep_inputs(inputs)
    res = run_bass_kernel_spmd(nc, in_maps, list(range(NCORES)), **kwargs)
    outs = [res.results[i]["y"] for i in range(NCORES)]   # each [2, BP]
    y = np.concatenate([o.T for o in outs], axis=0).astype(np.float32)  # [B, 2]
    return y, res


def kernel(**inputs):
    y, _ = run(inputs)
    return y


# revision 13
# speedup vs baseline: 1.2658x; 1.2658x over previous
"""Trainium2 Bass kernel for nn_BPModel: LSTM encoder -> latent ODE (RK4) -> decoder.

Data-parallel over 8 NeuronCores: batch 4096 -> 512 per core, all parameters
replicated, everything on-chip in [feature, batch] layout.

v2 design notes (from perfetto trace of v1):
- v1 was latency-bound on the LSTM recurrence chain (~6.9us/timestep) with
  ~350ns LDWEIGHTS per matmul: fp32r matmuls must self-load weights, so the
  walrus LDW-dedup pass could not elide anything.
- Weights (stationary operands) are now fp16: half-size loads, and walrus
  elides the second load of back-to-back matmuls sharing a stationary
  (emission keeps same-weight pairs adjacent). Moving operands stay fp32r
  (1 col/cycle at free-size >= 256) for precision.
- LSTM gates PSUM: per stream one [128, 1024] tile (2 banks), regions
  [i|f|o|g] at 256-col offsets: sigmoid(i,f,o) is ONE contiguous 768-col
  Act instr. 2 streams x 2 buffers = all 8 banks -> xproj(t+1) never waits.
- Elementwise split: t1=sig_i*tanh_g on Pool (gpsimd), t2/c'/h on DVE,
  tanh on Act. Whh matmuls skipped at t=0 (h=0).
- ODE: pn/cn trunks merged (stage1 one matmul via stacked [pn1W|cn1W]
  stationary + partition-stacked relu bias; stage2 two K=64 matmuls into one
  psum tile), one [3,256] pn3 matmul, ONE merged Exp with per-partition
  scale/bias APs, K=3 selector matmuls for row broadcasts, true (unpadded)
  weight dims, 2 independent batch streams, relu2 on DVE/Pool.

Engine instructions carry a single HW sync-wait slot; a post-Tile pass
moves excess waits onto same-engine NoOps.
"""

import sys
import numpy as np

for _p in ("/opt/trn_rl_repo",):
    if _p not in sys.path:
        sys.path.insert(0, _p)

import concourse.bass as bass
import concourse.tile as tile
import concourse.mybir as mybir
import concourse.bass_utils as _bu
from concourse.bass_utils import run_bass_kernel_spmd


def _patched_bir_verify_and_optimise(tmpdir, inp="bir.json", outp="file.neff",
                                     arch=None, *, dve_root=None):
    """Same as bass_utils.bir_verify_and_optimise but with walrus LDW
    dedup enabled (redundant LDWEIGHTS elision for back-to-back matmuls
    sharing a stationary operand)."""
    cmd = [
        _bu.get_walrus_driver(),
        "--pass",
        ",".join(["birverifier", "runtime_memory_reservation", "lower_act",
                  "lower_dve", "lower_ap_offset", "codegen", "neff_packager"]),
        "-i", inp,
        "--neff-output-filename", outp,
        "--enable-birsim=true", "--mem-mode=physical", "--policy=0",
        "--enable-ldw-opt=false",
        "--assign-static-dmas-to-sp=false",
        "--dram-page-size=256", "--enable-neff-debug-info=true",
        "--jobs", "8",
        *_bu.get_walrus_args(
            _bu.get_bir_arch(tmpdir, inp) if arch is None else arch,
            tmpdir, dve_root=dve_root),
    ]
    result = _bu.run_command(cmd, cwd=tmpdir)
    if result is not None:
        from pathlib import Path
        (Path(tmpdir) / "log.txt").write_text(result.stdout)
    return f"{tmpdir}/{outp}"


_bu.bir_verify_and_optimise = _patched_bir_verify_and_optimise

F32 = mybir.dt.float32
F32R = mybir.dt.float32r
F16 = mybir.dt.float16
AF = mybir.ActivationFunctionType
ALU = mybir.AluOpType

NCORES = 8
B, T_FULL, D_IN, H, LAT = 4096, 256, 2, 128, 128
BP = B // NCORES          # 512 batch per core
BS = BP // 2              # 256 per stream
N_STEPS = 9
SXT = 16                  # t-slots per xt3 tile (x rows 0..31, ones at 32)

# gate order in PSUM regions: i, f, o, g  (pytorch packs i, f, g, o)
GATE_PERM = (0, 1, 3, 2)
# whh/xproj emission order: g first so tanh(g) unblocks earliest
CI_ORDER = (3, 0, 1, 2)

# fp16 weight tensors (stationary matmul operands)
_W_SPECS = [
    ("Wball", [128, SXT * 512]),
    ("Whh", [128, 512]),
    ("fc1W", [128, 256]),
    ("W1stk", [128, 128]),     # [pn1W | cn1W]
    ("dec1aW", [128, 128]),
    ("fc2W", [128, 256]),
    ("W2stk", [128, 128]),     # rows 0:64 pn2W, 64:128 cn2W
    ("pn3W", [128, 3]),
    ("cn3W", [128, 128]),
    ("selS", [3, 128]),        # rows [1;1;0] -> bcast(rowA+rowB)
    ("selC", [3, 128]),        # rows [0;0;1] -> bcast(rowC)
    ("dec1bW3", [3, 128]),
    ("dec2W", [128, 64]),
    ("dec3W", [64, 2]),
]
# f32 bias / scale tensors
_B_SPECS = [
    ("fc1b2", [128, 2]),
    ("fc2b", [128, 1]),
    ("b1stk", [128, 1]),       # [pn1b(64); cn1b(64)]
    ("pn2b", [128, 1]), ("cn2b", [128, 1]),
    ("cn3b", [128, 1]),
    ("sc3", [3, 1]),           # exp scales [1, -1, -1]
    ("b3", [3, 1]),            # exp biases [b0, -b1, -b2]
    ("b3p", [3, 1]),           # params-exp biases [b0, b1, b2]
    ("dec1b", [128, 1]),
    ("dec2b", [64, 1]),
    ("dec3b", [2, 1]),
]


def _f32(ap):
    return ap.bitcast(F32)


def _legalize_matmul_waits(nc):
    """Engine instructions carry a single HW sync-wait slot (walrus: 'Too
    many sync wait commands'). Move excess waits onto preceding NoOps on the
    same engine queue; engine FIFO order keeps correctness."""
    n_moved = 0
    for fn in nc.m.functions:
        for bb in fn.blocks:
            out = []
            for inst in bb.instructions:
                si = inst.sync_info
                if si is not None and si.on_wait and len(si.on_wait) > 1:
                    waits = list(si.on_wait)
                    for w in waits[:-1]:
                        nop = mybir.InstNoOp(
                            name=nc.get_next_instruction_name(),
                            engine=inst.engine,
                            ins=[], outs=[],
                            sync_info=mybir.SyncInfo(on_wait=[w], on_update=[]),
                        )
                        out.append(nop)
                    si.on_wait = waits[-1:]
                    n_moved += 1
                out.append(inst)
            bb.instructions[:] = out
    return n_moved


def build_program(T=T_FULL, n_steps=N_STEPS, debug=False, legalize=True):
    dt = 1.0 / n_steps
    nxt = (T + SXT - 1) // SXT
    nc = bass.Bass()
    ins = {}
    ins["xt3"] = nc.declare_dram_parameter("xt3", [128, nxt * BP], F16,
                                           isOutput=False)
    for name, shape in _W_SPECS:
        ins[name] = nc.declare_dram_parameter(name, shape, F16, isOutput=False)
    for name, shape in _B_SPECS:
        ins[name] = nc.declare_dram_parameter(name, shape, F32, isOutput=False)
    y_out = nc.declare_dram_parameter("y", [2, BP], F32, isOutput=True)
    if debug:
        dbg_h = nc.declare_dram_parameter("dbg_h", [128, BP], F32, isOutput=True)
        dbg_z0 = nc.declare_dram_parameter("dbg_z0", [128, BP], F32, isOutput=True)
        dbg_zT = nc.declare_dram_parameter("dbg_zT", [128, BP], F32, isOutput=True)
        dbg_pr = nc.declare_dram_parameter("dbg_pr", [3, BP], F32, isOutput=True)

    with tile.TileContext(nc) as tc:
        with (
            tc.tile_pool(name="const", bufs=1) as cp,
            tc.tile_pool(name="state", bufs=2) as st,
        ):
            sb = {}
            sb["xt3"] = cp.tile([128, nxt * BP], F16, tag="xt3", name="xt3")
            nc.sync.dma_start(sb["xt3"][:], ins["xt3"][:])
            for name, shape in _W_SPECS:
                sb[name] = cp.tile(shape, F16, tag=name, name=name)
                nc.sync.dma_start(sb[name][:], ins[name][:])
            for name, shape in _B_SPECS:
                sb[name] = cp.tile(shape, F32, tag=name, name=name)
                nc.sync.dma_start(sb[name][:], ins[name][:])
            par3 = cp.tile([3, BP], F16, tag="par3")

            c = []
            for s in range(2):
                ct = st.tile([128, BS], F32, tag=f"c{s}")
                nc.gpsimd.memset(ct[:], 0.0)
                c.append(ct)

            xt3 = sb["xt3"]
            Wball = sb["Wball"]
            Whh = sb["Whh"]

            # ------------------ LSTM ------------------
            # per-stream gates psum [128, 2048]: one bank per gate [i|f|o|g],
            # 256 cols used of each 512-col bank; 2 streams = 8 banks.
            # g-gate weights are doubled host-side: tanh(g) = 2*sigmoid(2g)-1,
            # so ONE sigmoid instruction covers all four gate regions and the
            # correction runs as cheap DVE ops:
            #   u = 2*sig2g - 1 ; t1 = u*sig_i ; c' = t1 + sig_f*c
            with (
                tc.tile_pool(name="psA", bufs=1, space="PSUM") as gp,
                tc.tile_pool(name="work", bufs=3) as wp,
            ):
                h = [None, None]
                for t in range(T):
                    til, slot = divmod(t, SXT)
                    first = (t == 0)
                    gates = {}
                    for s in range(2):
                        gates[s] = gp.tile([128, 2048], F32, tag=f"g{s}",
                                           name=f"g{s}_{t}")
                    for ci in range(4):
                        for s in range(2):
                            xsl = xt3[:, BP * til + BS * s
                                      : BP * til + BS * (s + 1)]
                            nc.tensor.matmul(
                                gates[s][:, 512 * ci : 512 * ci + BS],
                                Wball[:, 512 * slot + 128 * ci
                                      : 512 * slot + 128 * (ci + 1)],
                                xsl, start=True, stop=first)
                    if not first:
                        for s in range(2):
                            for ci in range(4):
                                nc.tensor.matmul(
                                    gates[s][:, 512 * ci : 512 * ci + BS],
                                    Whh[:, 128 * ci : 128 * (ci + 1)],
                                    h[s][:], start=False, stop=True)
                    sgm = {}
                    for s in range(2):
                        sgm[s] = wp.tile([128, 4 * BS], F32, tag=f"sg{s}",
                                         name=f"sg{s}_{t}")
                        ga = gates[s][:].rearrange("p (r q) -> p r q", r=4)
                        nc.scalar.activation(sgm[s][:], ga[:, 0:4, 0:BS],
                                             AF.Sigmoid)
                    t2 = {}
                    u = {}
                    t1 = {}
                    for s in range(2):
                        # off-path: t2 = sig_f * c on Pool
                        t2[s] = wp.tile([128, BS], F32, tag=f"t2{s}",
                                        name=f"t2{s}_{t}")
                        nc.gpsimd.tensor_tensor(
                            out=t2[s][:], in0=sgm[s][:, BS : 2 * BS],
                            in1=c[s][:], op=ALU.mult)
                    for s in range(2):
                        u[s] = wp.tile([128, BS], F32, tag=f"u{s}",
                                       name=f"u{s}_{t}")
                        nc.vector.tensor_scalar(
                            out=u[s][:], in0=sgm[s][:, 3 * BS : 4 * BS],
                            scalar1=2.0, scalar2=1.0,
                            op0=ALU.mult, op1=ALU.subtract)
                        t1[s] = wp.tile([128, BS], F32, tag=f"t1{s}",
                                        name=f"t1{s}_{t}")
                        nc.vector.tensor_tensor(
                            out=t1[s][:], in0=u[s][:], in1=sgm[s][:, 0:BS],
                            op=ALU.mult)
                    cn = {}
                    for s in range(2):
                        cn[s] = st.tile([128, BS], F32, tag=f"c{s}",
                                        name=f"c{s}_{t}")
                        nc.vector.tensor_tensor(
                            out=cn[s][:], in0=t1[s][:], in1=t2[s][:],
                            op=ALU.add)
                        c[s] = cn[s]
                    tct = {}
                    for s in range(2):
                        tct[s] = wp.tile([128, BS], F32, tag=f"tc{s}",
                                         name=f"tc{s}_{t}")
                        nc.scalar.activation(tct[s][:], cn[s][:], AF.Tanh)
                    for s in range(2):
                        hn_ = st.tile([128, BS], F16, tag=f"h{s}",
                                      name=f"h{s}_{t}")
                        nc.vector.tensor_tensor(
                            out=hn_[:], in0=sgm[s][:, 2 * BS : 3 * BS],
                            in1=tct[s][:], op=ALU.mult)
                        h[s] = hn_

            # ------------- encoder fc + ODE + decoder -------------
            with (
                tc.tile_pool(name="psB", bufs=1, space="PSUM") as pb,
                tc.tile_pool(name="ow", bufs=2) as ow,
            ):
                if debug:
                    for s in range(2):
                        nc.sync.dma_start(
                            dbg_h[:, BS * s : BS * (s + 1)].bitcast(F16)[:, 0:BS],
                            h[s][:])
                # fc1: relu(hN @ fc1W + b); j chunks of the 256-dim output
                r1 = ow.tile([128, 1024], F16, tag="r1")
                for j in range(2):
                    pfc = pb.tile([128, 512], F32, tag=f"pA{j}", name=f"pfc{j}")
                    for s in range(2):
                        nc.tensor.matmul(
                            pfc[:, BS * s : BS * (s + 1)],
                            sb["fc1W"][:, 128 * j : 128 * (j + 1)],
                            h[s][:], start=(s == 0), stop=(s == 1))
                    nc.scalar.activation(
                        r1[:, 512 * j : 512 * (j + 1)], pfc[:], AF.Relu,
                        bias=sb["fc1b2"][:, j : j + 1])
                # fc2 (no relu)
                pz = pb.tile([128, BP], F32, tag="pB0")
                nc.tensor.matmul(pz[:], sb["fc2W"][:, 0:128], r1[:, 0:512],
                                 start=True, stop=False)
                nc.tensor.matmul(pz[:], sb["fc2W"][:, 128:256], r1[:, 512:1024],
                                 start=False, stop=True)
                zs = []
                for s in range(2):
                    zt = ow.tile([128, BS], F16, tag=f"z{s}")
                    nc.vector.tensor_scalar(
                        out=zt[:], in0=pz[:, BS * s : BS * (s + 1)],
                        scalar1=sb["fc2b"][:], scalar2=None, op0=ALU.add)
                    zs.append(zt)
                if debug:
                    for s in range(2):
                        nc.sync.dma_start(dbg_z0[:, BS * s : BS * (s + 1)].bitcast(F16)[:, 0:BS], zs[s][:])

                def odef(zin, s, first=False, ktag="k"):
                    """One odefunc eval for stream s: k = (comp + cn3b
                    - z*(Rp + 1/Rd)) / C, trunks merged. Each matmul output
                    gets its own psum bank (tags pA..pD cycle per stream)."""
                    sl = slice(BS * s, BS * (s + 1))
                    # stage 1: both trunks in one matmul (partition-stacked)
                    p1 = pb.tile([128, 512], F32, tag=f"pA{s}", name=f"p1{s}")
                    nc.tensor.matmul(p1[:, 0:BS], sb["W1stk"][:], zin[:],
                                     start=True, stop=True)
                    s1 = ow.tile([128, BS], F16, tag=f"s1_{s}")
                    nc.scalar.activation(s1[:], p1[:, 0:BS], AF.Relu,
                                         bias=sb["b1stk"][:])
                    # stage 2: two K=64 matmuls, separate banks
                    p2a = pb.tile([128, 512], F32, tag=f"pB{s}", name=f"p2a{s}")
                    nc.tensor.matmul(p2a[:, 0:BS], sb["W2stk"][0:64, :],
                                     s1[0:64, :], start=True, stop=True)
                    p2b = pb.tile([128, 512], F32, tag=f"pC{s}", name=f"p2b{s}")
                    nc.tensor.matmul(p2b[:, 0:BS], sb["W2stk"][64:128, :],
                                     s1[64:128, :], start=True, stop=True)
                    s2p = ow.tile([128, BS], F16, tag=f"s2p{s}")
                    nc.scalar.activation(s2p[:], p2a[:, 0:BS], AF.Relu,
                                         bias=sb["pn2b"][:])
                    s2c = ow.tile([128, BS], F16, tag=f"s2c{s}")
                    nc.scalar.activation(s2c[:], p2b[:, 0:BS], AF.Relu,
                                         bias=sb["cn2b"][:])
                    # stage 3: pn3 -> [3, BS] (bank A); cn3 -> [128, BS] (bank B)
                    p3 = pb.tile([128, 512], F32, tag=f"pA{s}", name=f"p3{s}")
                    pp3 = p3[0:3, 0:BS]
                    nc.tensor.matmul(pp3, sb["pn3W"][:], s2p[:],
                                     start=True, stop=True)
                    pcn = pb.tile([128, 512], F32, tag=f"pB{s}", name=f"pcn{s}")
                    nc.tensor.matmul(pcn[:, 0:BS], sb["cn3W"][:], s2c[:],
                                     start=True, stop=True)
                    # rows = [Rp; 1/Rd; 1/C] = exp(pp3 * [1,-1,-1] + [b0,-b1,-b2])
                    rows = ow.tile([3, BS], F16, tag=f"rw{s}")
                    nc.scalar.activation(rows[:], pp3, AF.Exp,
                                         bias=sb["b3"][:], scale=sb["sc3"][:])
                    if first:
                        nc.scalar.activation(par3[:, sl], pp3, AF.Exp,
                                             bias=sb["b3p"][:], scale=1.0)
                    # Sb = bcast(Rp + 1/Rd) (bank D); Cb = bcast(1/C) (bank C)
                    pbs = pb.tile([128, 512], F32, tag=f"pD{s}", name=f"pbs{s}")
                    nc.tensor.matmul(pbs[:, 0:BS], sb["selS"][:], rows[:],
                                     start=True, stop=True)
                    pbc = pb.tile([128, 512], F32, tag=f"pC{s}", name=f"pbc{s}")
                    nc.tensor.matmul(pbc[:, 0:BS], sb["selC"][:], rows[:],
                                     start=True, stop=True)
                    # k = (comp + cn3b - z*Sb) * Cb
                    d1 = ow.tile([128, BS], F32, tag=f"d1{s}")
                    nc.vector.tensor_tensor(out=d1[:], in0=zin[:],
                                            in1=pbs[:, 0:BS], op=ALU.mult)
                    d2 = ow.tile([128, BS], F32, tag=f"d2{s}")
                    nc.vector.scalar_tensor_tensor(
                        out=d2[:], in0=pcn[:, 0:BS], scalar=sb["cn3b"][:],
                        in1=d1[:], op0=ALU.add, op1=ALU.subtract)
                    k = ow.tile([128, BS], F32, tag=ktag)
                    nc.vector.tensor_tensor(out=k[:], in0=d2[:],
                                            in1=pbc[:, 0:BS], op=ALU.mult)
                    return k

                def sttz(k_in0, scalar, ztile, tag):
                    # f16 out: (k * scalar) + z
                    o = ow.tile([128, BS], F16, tag=tag)
                    nc.vector.scalar_tensor_tensor(
                        out=o[:], in0=k_in0[:], scalar=float(scalar),
                        in1=ztile[:],
                        op0=ALU.mult, op1=ALU.add)
                    return o

                def sttk(in0, scalar, in1, tag):
                    # f32 out: (in0 * scalar) + in1
                    o = ow.tile([128, BS], F32, tag=tag)
                    nc.vector.scalar_tensor_tensor(
                        out=o[:], in0=in0[:], scalar=float(scalar), in1=in1[:],
                        op0=ALU.mult, op1=ALU.add)
                    return o

                def ttp(in0, in1, op, tag):
                    # f32 out on Pool
                    o = ow.tile([128, BS], F32, tag=tag)
                    nc.gpsimd.tensor_tensor(out=o[:], in0=in0[:], in1=in1[:],
                                            op=op)
                    return o

                for step in range(n_steps):
                    for s in range(2):
                        z = zs[s]
                        k1 = odef(z, s, first=(step == 0), ktag=f"k1{s}")
                        za = sttz(k1, dt / 3.0, z, f"za{s}")   # z + dt/3 k1
                        k2 = odef(za, s, ktag=f"k2{s}")
                        u1 = sttk(k1, -1.0 / 3.0, k2, f"u1{s}")  # k2 - k1/3
                        zb = sttz(u1, dt, z, f"za{s}")  # z + dt(k2 - k1/3)
                        k3 = odef(zb, s, ktag=f"k3{s}")
                        u2 = ttp(k1, k2, ALU.subtract, f"u1{s}")
                        u3 = ttp(u2, k3, ALU.add, f"u2{s}")
                        zc2 = sttz(u3, dt, z, f"za{s}")  # z + dt(k1 - k2 + k3)
                        k4 = odef(zc2, s, ktag=f"k4{s}")
                        v1 = ttp(k2, k3, ALU.add, f"u1{s}")
                        v2 = sttk(v1, 3.0, k1, f"u2{s}")  # k1 + 3(k2 + k3)
                        v3 = ttp(v2, k4, ALU.add, f"u1{s}")
                        zs[s] = sttz(v3, dt / 8.0, z, f"z{s}")  # z + dt/8 (..)

                for s in range(2):
                    sl = slice(BS * s, BS * (s + 1))
                    if debug:
                        nc.sync.dma_start(dbg_zT[:, sl].bitcast(F16)[:, 0:BS], zs[s][:])
                        if s == 0:
                            nc.sync.dma_start(dbg_pr[:].bitcast(F16)[:, 0:BP], par3[:])
                    # decoder: zc = [zT ; params]
                    pd1 = pb.tile([128, 512], F32, tag=f"pA{s}",
                                  name=f"pd1{s}")
                    nc.tensor.matmul(pd1[:, 0:BS], sb["dec1aW"][:], zs[s][:],
                                     start=True, stop=False)
                    nc.tensor.matmul(pd1[:, 0:BS], sb["dec1bW3"][:],
                                     par3[:, sl], start=False, stop=True)
                    sd1 = ow.tile([128, BS], F16, tag=f"sd1{s}")
                    nc.scalar.activation(sd1[:], pd1[:, 0:BS], AF.Relu,
                                         bias=sb["dec1b"][:])
                    pd2 = pb.tile([128, 512], F32, tag=f"pB{s}",
                                  name=f"pd2{s}")
                    nc.tensor.matmul(pd2[0:64, 0:BS], sb["dec2W"][:], sd1[:],
                                     start=True, stop=True)
                    sd2 = ow.tile([64, BS], F16, tag=f"sd2{s}")
                    nc.scalar.activation(sd2[:], pd2[0:64, 0:BS], AF.Relu,
                                         bias=sb["dec2b"][:])
                    pd3 = pb.tile([128, 512], F32, tag=f"pC{s}",
                                  name=f"pd3{s}")
                    nc.tensor.matmul(pd3[0:2, 0:BS], sb["dec3W"][:], sd2[:],
                                     start=True, stop=True)
                    yt = ow.tile([2, BS], F32, tag=f"y{s}")
                    nc.vector.tensor_scalar(out=yt[:], in0=pd3[0:2, 0:BS],
                                            scalar1=sb["dec3b"][:],
                                            scalar2=None, op0=ALU.add)
                    nc.sync.dma_start(y_out[:, sl], yt[:])

    if legalize:
        _legalize_matmul_waits(nc)
    return nc


def prep_inputs(inputs, T=T_FULL):
    """Host-side marshaling: shard x, build xt3/Wball layouts, repack weights."""
    nxt = (T + SXT - 1) // SXT
    f = lambda a: np.ascontiguousarray(a, dtype=np.float32)
    f16 = lambda a: np.ascontiguousarray(a, dtype=np.float16)
    x = f(inputs["x"])                      # [B, T, 2]
    Wih = f(inputs["lstm_Wih"])             # [2, 512]
    Whh = f(inputs["lstm_Whh"])             # [128, 512]
    bsum = f(inputs["lstm_bih"] + inputs["lstm_bhh"])   # [512]

    # permute gate chunks (i, f, g, o) -> (i, f, o, g)
    def permc(w):
        chunks = [w[..., 128 * cc : 128 * (cc + 1)] for cc in GATE_PERM]
        return np.concatenate(chunks, axis=-1)

    Wih_p, Whh_p, bsum_p = permc(Wih), permc(Whh), permc(bsum)
    # g-gate doubled: tanh(g) computed as 2*sigmoid(2g)-1 on-chip
    Wih_p[:, 384:512] *= 2.0
    Whh_p[:, 384:512] *= 2.0
    bsum_p[384:512] *= 2.0

    # Wball: [128, SXT*512]; slot s: rows 2s,2s+1 = Wih rows, row 32 = bias
    Wball = np.zeros((128, SXT * 512), dtype=np.float32)
    for s in range(SXT):
        Wball[2 * s, 512 * s : 512 * (s + 1)] = Wih_p[0]
        Wball[2 * s + 1, 512 * s : 512 * (s + 1)] = Wih_p[1]
        Wball[32, 512 * s : 512 * (s + 1)] = bsum_p

    # xt3 per core: [128, nxt*BP]; tile t//SXT, x rows 2(t%SXT), ones row 32
    xt3_all = np.zeros((NCORES, 128, nxt * BP), dtype=np.float16)
    xs = x.reshape(NCORES, BP, T, 2)
    for core in range(NCORES):
        xc = xs[core]                       # [BP, T, 2]
        for t in range(T):
            til, slot = divmod(t, SXT)
            col0 = BP * til
            xt3_all[core, 2 * slot, col0 : col0 + BP] = xc[:, t, 0]
            xt3_all[core, 2 * slot + 1, col0 : col0 + BP] = xc[:, t, 1]
        xt3_all[core, 32, :] = 1.0

    fc1_b = f(inputs["fc1_b"])
    fc2_W = f(inputs["fc2_W"])
    pn3_b = f(inputs["pn3_b"])
    dec1_W = f(inputs["dec1_W"])            # [131, 128]

    selS = np.zeros((3, 128), dtype=np.float32)
    selS[0, :] = 1.0
    selS[1, :] = 1.0
    selC = np.zeros((3, 128), dtype=np.float32)
    selC[2, :] = 1.0

    common = {
        "Wball": f16(Wball),
        "Whh": f16(Whh_p),
        "fc1W": f16(inputs["fc1_W"]),
        "fc1b2": f(fc1_b.reshape(2, 128).T),
        "fc2W": f16(np.concatenate([fc2_W[0:128], fc2_W[128:256]], axis=1)),
        "fc2b": f(inputs["fc2_b"][:, None]),
        "W1stk": f16(np.concatenate(
            [inputs["pn1_W"], inputs["cn1_W"]], axis=1)),   # [128, 64+64]
        "b1stk": f(np.concatenate(
            [inputs["pn1_b"], inputs["cn1_b"]])[:, None]),
        "W2stk": f16(np.concatenate(
            [inputs["pn2_W"], inputs["cn2_W"]], axis=0)),  # [128, 128]
        "pn2b": f(inputs["pn2_b"][:, None]),
        "cn2b": f(inputs["cn2_b"][:, None]),
        "pn3W": f16(inputs["pn3_W"]),        # [128, 3]
        "cn3W": f16(inputs["cn3_W"]),        # [128, 128]
        "cn3b": f(inputs["cn3_b"][:, None]),
        "sc3": np.array([[1.0], [-1.0], [-1.0]], dtype=np.float32),
        "b3": np.array([[pn3_b[0]], [-pn3_b[1]], [-pn3_b[2]]],
                       dtype=np.float32),
        "b3p": f(pn3_b[:, None]),
        "selS": f16(selS),
        "selC": f16(selC),
        "dec1aW": f16(dec1_W[0:128]),
        "dec1bW3": f16(dec1_W[128:131]),
        "dec1b": f(inputs["dec1_b"][:, None]),
        "dec2W": f16(inputs["dec2_W"]),      # [128, 64]
        "dec2b": f(inputs["dec2_b"][:, None]),
        "dec3W": f16(inputs["dec3_W"]),      # [64, 2]
        "dec3b": f(inputs["dec3_b"][:, None]),
    }

    in_maps = []
    for core in range(NCORES):
        m = dict(common)
        m["xt3"] = xt3_all[core]
        in_maps.append(m)
    return in_maps


_PROGRAM = None


def get_program():
    global _PROGRAM
    if _PROGRAM is None:
        _PROGRAM = build_program()
    return _PROGRAM


def run(inputs, **kwargs):
    nc = get_program()
    in_maps = prep_inputs(inputs)
    res = run_bass_kernel_spmd(nc, in_maps, list(range(NCORES)), **kwargs)
    outs = [res.results[i]["y"] for i in range(NCORES)]   # each [2, BP]
    y = np.concatenate([o.T for o in outs], axis=0).astype(np.float32)  # [B, 2]
    return y, res


def kernel(**inputs):
    y, _ = run(inputs)
    return y
